# revision 22
# baseline (speedup 1.0000x reference)
# GraphSAGE (3-layer, mean aggregation) on 8 Trainium2 NeuronCores.
#
# Sharding: nodes are split into 8 contiguous ranges (6250 per core); edges are
# partitioned by destination node so each core's scatter-adds stay local.  Each
# layer's input features are replicated to every core via AllGather (the x table
# for layer 0 is simply fed to every core), so the per-edge source gathers are
# local HBM reads.
#
# All tables / weights / PE operands are fp16 (full-rate on the PE, 2x on the
# DVE, and ~8x the mantissa precision of bf16).
#
# Gathers issue 512-index SWDGE instructions round-robin over 4 queues
# (measured ~2x on hardware vs one queue; the descriptor-generation ucode
# parallelizes across queues), with per-core true index counts passed through
# a register so SPMD padding generates no descriptors.  Edge slots are sorted
# by source row within each segment for HBM locality.
#
# The boundary after layer 1 carries g = relu(h1) @ wl2 (64-wide, packed two
# nodes per 256B gather row, selected by per-edge parity masks), halving that
# AllGather; layer 1 produces feature-major relu(h1) directly (transposed
# matmuls) so layer 2 needs no per-chunk PE transpose.
#
# Each layer runs in TWO PASSES so the inter-layer AllGathers overlap compute:
#   pass A: for each chunk of 128 destination nodes, gather the edges whose
#     sources live in the LOW half of the feature table and accumulate the
#     transposed partial mean  loPart[c][feat, dst]  on the PE (via per-tile
#     selection matrices S[e, j] = (dst_local[e] == j) / deg built on the DVE),
#     parking the partial in SBUF.
#   pass B: gather the HIGH-half edges, accumulate hiPart, then
#     h = relu(loPart^T w_l + hiPart^T w_l + own^T^T w_r (+ b)) with three
#     accumulating matmuls into one PSUM tile; the Activation engine applies
#     relu / drains PSUM copies.
# The low-half AllGather of layer l's output fires mid-way through pass B
# (after the chunks that produce table rows [0, HALFR) complete), and the
# high-half AllGather's latency is hidden by layer l+1's pass A, which only
# needs the low half.
#
# Gather slots are padded (SPMD-uniform tile counts) with index -1: the SWDGE
# ucode skips trailing negative indices, so padding costs no descriptors/HBM
# bandwidth. Selection-matrix columns for padded slots are 0 (dst_local = -1),
# so whatever data sits in the padded SBUF slots contributes nothing.
import os
import sys

import numpy as np

for _p in ("/opt/trn_rl_repo", "/root/.axon_site/_ro/trn_rl_repo"):
    if _p not in sys.path and os.path.isdir(_p):
        sys.path.append(_p)

from concourse import bacc, mybir, tile  # noqa: E402
from concourse.bass_utils import axon_active, run_bass_kernel_spmd  # noqa: E402
from concourse.masks import make_identity  # noqa: E402

P = 128
SORT_SLOTS = True  # sort gather slots by source row within each segment
FP16 = mybir.dt.float16
F32 = mybir.dt.float32
I16 = mybir.dt.int16
NP_FP16 = np.float16
MAXI = 512  # gather indices per SWDGE instruction (descriptor ring is 1024)


class GSCfg:
    """Static problem configuration (shapes shared by all cores)."""

    def __init__(self, n_nodes, n_cores, d_in, d_hid, d_out, half, maxi=MAXI):
        assert n_nodes % n_cores == 0
        self.MAXI = maxi
        self.N = n_nodes
        self.NCORES = n_cores
        self.NPC = n_nodes // n_cores  # nodes per core
        self.D_IN = d_in
        self.D_HID = d_hid
        self.D_OUT = d_out
        # Gather tables are split at row `half` so int16 indices can address
        # each piece. half % n_cores == 0; per-core producer split = HALFR.
        self.HALF = half
        assert half % n_cores == 0
        self.HALFR = half // n_cores
        assert self.HALFR < self.NPC
        assert half <= 2**15 and (n_nodes - half) < 2**15
        self.NCH = (self.NPC + P - 1) // P  # dst chunks per core


def preprocess(cfg: GSCfg, src: np.ndarray, dst: np.ndarray):
    """Partition + sort edges by destination, pad to an SPMD-uniform tile
    layout, and build the per-core SBUF-layout index/selection arrays."""
    N, NPC, NCH, HALF, HALFR = cfg.N, cfg.NPC, cfg.NCH, cfg.HALF, cfg.HALFR
    NC = cfg.NCORES

    deg = np.bincount(dst, minlength=N)
    invdeg_per_node = (1.0 / np.maximum(deg, 1)).astype(np.float32)

    order = np.argsort(dst, kind="stable")
    s_src = src[order]
    s_dst = dst[order]

    # edge index ranges for (core, chunk)
    chunk_lo = np.empty((NC, NCH), dtype=np.int64)
    chunk_hi = np.empty((NC, NCH), dtype=np.int64)
    for i in range(NC):
        for c in range(NCH):
            d0 = i * NPC + c * P
            d1 = i * NPC + min((c + 1) * P, NPC)
            chunk_lo[i, c] = np.searchsorted(s_dst, d0, side="left")
            chunk_hi[i, c] = np.searchsorted(s_dst, d1, side="left")

    # split each chunk's edges into low-src / high-src groups
    lo_cnt = np.zeros((NC, NCH), dtype=np.int64)
    for i in range(NC):
        for c in range(NCH):
            e0, e1 = chunk_lo[i, c], chunk_hi[i, c]
            lo_cnt[i, c] = int(np.count_nonzero((s_src[e0:e1] % NPC) < HALFR))
    hi_cnt = (chunk_hi - chunk_lo) - lo_cnt

    cdiv = lambda a, b: -(-a // b)
    T_low = [int(cdiv(int(lo_cnt[:, c].max()), P)) for c in range(NCH)]
    T_high = [int(cdiv(int(hi_cnt[:, c].max()), P)) for c in range(NCH)]
    TT = sum(T_low) + sum(T_high)
    SLOTS = TT * P

    # Padded slots carry index -1: the SWDGE ucode generates no descriptors
    # for a trailing negative run, and each core passes its true per-window
    # count through a register (num_idxs_reg), so padding costs no HBM
    # bandwidth.  Windows that are entirely padding keep one real index
    # (row 0, count 1) so the completion path always sees >= 1 descriptor.
    # dstloc=-1 masks padded slots out of the selection matrices either way.
    idx16 = np.full((NC, SLOTS), -1, dtype=np.int16)
    idx16_l2 = np.full((NC, SLOTS), -1, dtype=np.int16)  # packed-pair rows
    dstloc = np.full((NC, SLOTS), -1.0, dtype=np.float32)
    invd = np.zeros((NC, SLOTS), dtype=np.float32)
    inve = np.zeros((NC, SLOTS), dtype=np.float32)  # invd if even table row
    invo = np.zeros((NC, SLOTS), dtype=np.float32)  # invd if odd table row
    MX = cfg.MAXI
    n_win = sum(-(-(t * P) // MX) for t in T_low + T_high)
    gcnt = np.ones((NC, n_win), dtype=np.int32)
    # windows full on every core keep a compile-time count (no reg_load)
    win_full = np.ones(n_win, dtype=bool)

    # Layer order within the slot array: all chunks' LOW segments first
    # (pass A), then all chunks' HIGH segments (pass B).
    HIR = NPC - HALFR
    for i in range(NC):
        seg_data = {}
        for c in range(NCH):
            e0, e1 = chunk_lo[i, c], chunk_hi[i, c]
            seg_src = s_src[e0:e1]
            seg_dst = s_dst[e0:e1]
            s_i, s_r = seg_src // NPC, seg_src % NPC
            is_lo = s_r < HALFR
            tidx = np.where(is_lo, s_i * HALFR + s_r, s_i * HIR + (s_r - HALFR))
            base = i * NPC + c * P
            for half_sel in (0, 1):
                m = is_lo if half_sel == 0 else ~is_lo
                # source-sorted slot order: consecutive gather descriptors hit
                # nearby table rows (HBM row-buffer locality); any order within
                # a segment is valid since dstloc/invd follow the slot
                if SORT_SLOTS:
                    o = np.argsort(tidx[m], kind="stable")
                else:
                    o = np.arange(int(np.count_nonzero(m)))
                seg_data[(half_sel, c)] = (
                    tidx[m][o].astype(np.int16),
                    (seg_dst[m][o] - base).astype(np.float32),
                    invdeg_per_node[seg_dst[m][o]],
                )
        pos = 0
        win = 0
        for half_sel in (0, 1):
            T_pads = T_low if half_sel == 0 else T_high
            for c in range(NCH):
                ss, dd, vv = seg_data[(half_sel, c)]
                n = len(ss)
                t_pad = T_pads[c]
                assert n <= t_pad * P
                idx16[i, pos : pos + n] = ss
                idx16_l2[i, pos : pos + n] = ss // 2
                dstloc[i, pos : pos + n] = dd
                invd[i, pos : pos + n] = vv
                par = (ss & 1).astype(np.float32)
                inve[i, pos : pos + n] = vv * (1.0 - par)
                invo[i, pos : pos + n] = vv * par
                for w0 in range(0, t_pad * P, MX):
                    wlen = min(MX, t_pad * P - w0)
                    real = min(max(n - w0, 0), wlen)
                    if real == 0:
                        idx16[i, pos + w0] = 0
                        idx16_l2[i, pos + w0] = 0
                        real = 1
                    if real < wlen:
                        win_full[win] = False
                    gcnt[i, win] = real
                    win += 1
                pos += t_pad * P
        assert pos == SLOTS and win == n_win

    # SBUF layouts:
    #  idx16_sb [128, SLOTS//16]: per gather segment, slot j -> [j%16, j//16],
    #    replicated across the eight 16-partition groups.  Segments are
    #    multiples of 128 slots, so the per-segment wrap equals a global wrap.
    def idx_layout(a):
        w = a.reshape(NC, SLOTS // 16, 16).transpose(0, 2, 1)  # [NC,16,cols]
        return np.ascontiguousarray(np.tile(w, (1, 8, 1)))  # [NC,128,cols]

    #  dstloc/invd [128, TT]: slot j -> [j%128, j//128]
    def col_layout(a):
        return np.ascontiguousarray(a.reshape(NC, TT, P).transpose(0, 2, 1))

    #  gcnt [128, ceil(n_win/128)]: window w -> [w%128, w//128]
    wcols = -(-n_win // P)
    gcnt_sb = np.zeros((NC, P, wcols), dtype=np.int32)
    for w in range(n_win):
        gcnt_sb[:, w % P, w // P] = gcnt[:, w]

    return (
        T_low,
        T_high,
        idx_layout(idx16),
        col_layout(dstloc),
        col_layout(invd),
        idx_layout(idx16_l2),
        col_layout(inve),
        col_layout(invo),
        gcnt_sb,
        win_full,
    )


def table_permute(cfg: GSCfg, x: np.ndarray) -> np.ndarray:
    """Reorder node rows into the core-major-half gather-table layout."""
    g = np.arange(cfg.N)
    i, r = g // cfg.NPC, g % cfg.NPC
    hr, hir = cfg.HALFR, cfg.NPC - cfg.HALFR
    gp = np.where(r < hr, i * hr + r, cfg.HALF + i * hir + (r - hr))
    out = np.empty_like(x)
    out[gp] = x[g]
    return out


def build_program(cfg: GSCfg, T_low, T_high, has_bias, skip_collectives=False,
                  repeat=1, n_queues=4, ablate=(), msg_bufs=8, sel_bufs=16):
    """Build the SPMD Bass program (identical instruction stream per core).

    repeat>1 runs the whole 3-layer computation that many times inside one
    NEFF (constants stay resident); used by the benchmark harness to measure
    per-iteration device time as a slope, independent of dispatch overhead."""
    assert not has_bias, "bias path not implemented (reference biases are zero)"

    N, NPC, NCH, HALF, HALFR = cfg.N, cfg.NPC, cfg.NCH, cfg.HALF, cfg.HALFR
    D_IN, D_HID, D_OUT = cfg.D_IN, cfg.D_HID, cfg.D_OUT
    TT = sum(T_low) + sum(T_high)
    TMAX = max(max(T_low), max(T_high))
    HIR = NPC - HALFR
    # pass-B chunk index after which all of h_own[0:HALFR] has been written
    C_LOW_DONE = (HALFR + P - 1) // P - 1

    nc = bacc.Bacc(
        "TRN2",
        target_bir_lowering=False,
        debug=not axon_active(),
        num_devices=cfg.NCORES,
        num_swdge_queues=n_queues,
        dynamic_dma_scratch_size=max(16384, 32 * cfg.MAXI),
    )
    maxi = cfg.MAXI
    n_win = sum(-(-(t * P) // maxi) for t in T_low + T_high)
    wcols = -(-n_win // P)
    win_full = getattr(cfg, "win_full", None)

    xtab = nc.dram_tensor("xtab", [N, D_IN], FP16, kind="ExternalInput")
    # feature-major own rows (precomputed on host): layer 0's root term reads
    # columns directly, skipping the per-chunk PE transpose + PSUM drain
    xownT = nc.dram_tensor("xownT", [D_IN, NPC], FP16, kind="ExternalInput")
    idx_d = nc.dram_tensor("idx16", [P, TT * 8], I16, kind="ExternalInput")
    dst_d = nc.dram_tensor("dstloc", [P, TT], F32, kind="ExternalInput")
    inv_d = nc.dram_tensor("invd", [P, TT], F32, kind="ExternalInput")
    idx2_d = nc.dram_tensor("idx16l2", [P, TT * 8], I16, kind="ExternalInput")
    inve_d = nc.dram_tensor("inve", [P, TT], F32, kind="ExternalInput")
    invo_d = nc.dram_tensor("invo", [P, TT], F32, kind="ExternalInput")
    gcnt_d = nc.dram_tensor("gcnt", [P, wcols], mybir.dt.int32, kind="ExternalInput")
    w_d = {}
    for li, (din, dout) in enumerate(((D_IN, D_HID), (D_HID, D_HID), (D_HID, D_OUT))):
        w_d[f"wl{li}"] = nc.dram_tensor(f"wl{li}", [din, dout], FP16, kind="ExternalInput")
        w_d[f"wr{li}"] = nc.dram_tensor(f"wr{li}", [din, dout], FP16, kind="ExternalInput")

    out_d = nc.dram_tensor("out", [NPC, D_OUT], F32, kind="ExternalOutput")

    from contextlib import ExitStack

    with tile.TileContext(nc) as tc, ExitStack() as stk:
        # ---- constants / static SBUF residents ----
        const = stk.enter_context(tc.tile_pool(name="const", bufs=1))
        iota_i = const.tile([P, P], mybir.dt.int32, name="iota_i")
        nc.gpsimd.iota(iota_i[:], pattern=[[1, P]], base=0, channel_multiplier=0)
        iota_h = const.tile([P, P], FP16, name="iota_h")
        nc.vector.tensor_copy(iota_h[:], iota_i[:])
        ident = const.tile([P, P], FP16, name="ident")
        make_identity(nc, ident[:])

        idx_t = const.tile([P, TT * 8], I16, name="idx_t")
        nc.sync.dma_start(idx_t[:], idx_d[:])
        dst_t = const.tile([P, TT], F32, name="dst_t")
        nc.sync.dma_start(dst_t[:], dst_d[:])
        inv_t = const.tile([P, TT], F32, name="inv_t")
        nc.sync.dma_start(inv_t[:], inv_d[:])
        idx2_t = const.tile([P, TT * 8], I16, name="idx2_t")
        nc.sync.dma_start(idx2_t[:], idx2_d[:])
        inve_t = const.tile([P, TT], F32, name="inve_t")
        nc.sync.dma_start(inve_t[:], inve_d[:])
        invo_t = const.tile([P, TT], F32, name="invo_t")
        nc.sync.dma_start(invo_t[:], invo_d[:])
        gcnt_t = const.tile([P, wcols], mybir.dt.int32, name="gcnt_t")
        nc.sync.dma_start(gcnt_t[:], gcnt_d[:])
        cnt_reg = nc.gpsimd.alloc_register("gather_cnt")

        w_t = {}
        for k, d in w_d.items():
            w_t[k] = const.tile(list(d.shape), d.dtype, name=f"{k}_t")
            nc.sync.dma_start(w_t[k][:], d[:])

        # per-chunk low-half partial aggregations (persistent across a layer)
        lop = stk.enter_context(tc.tile_pool(name="lop", bufs=1))
        loPart = [lop.tile([P, P], FP16, name=f"loPart{c}") for c in range(NCH)]

        # ---- inter-layer DRAM tables ----
        # (allocated per repetition: Shared collective outputs allow only a
        # single writer instruction)
        dram = stk.enter_context(tc.tile_pool(name="dram", bufs=1, space="DRAM"))

        def alloc_tables(rep):
            # layer-0 output: node-major h0 rows + its AllGather halves
            h0_own = dram.tile([NPC, D_HID], FP16, name=f"h0_own_r{rep}")
            h0_full = (
                dram.tile([HALF, D_HID], FP16, name=f"h0_full0_r{rep}",
                          addr_space="Shared"),
                dram.tile([N - HALF, D_HID], FP16, name=f"h0_full1_r{rep}",
                          addr_space="Shared"),
            )
            # layer-1 outputs: g = relu(h1) @ wl2 rows (node-major, feeds the
            # boundary-1 AllGather; consumed packed two-nodes-per-row) and
            # feature-major relu(h1) for layer 2's root term
            g_own = dram.tile([NPC, D_OUT], FP16, name=f"g_own_r{rep}")
            h1T_own = dram.tile([P, NPC], FP16, name=f"h1T_own_r{rep}")
            g_full = (
                dram.tile([HALF // 2, P], FP16, name=f"g_full0_r{rep}",
                          addr_space="Shared"),
                dram.tile([(N - HALF) // 2, P], FP16, name=f"g_full1_r{rep}",
                          addr_space="Shared"),
            )
            return h0_own, h0_full, g_own, h1T_own, g_full

        # ---- working pools ----
        msgp = stk.enter_context(tc.tile_pool(name="msg", bufs=msg_bufs))
        sp = stk.enter_context(tc.tile_pool(name="sel", bufs=sel_bufs))
        # 4-deep working pool: all six pass-B tags (hi/xT/own/h/g/o) rotate
        # one chunk deeper, letting combine/drain stages of consecutive
        # chunks overlap the aggregation pipeline
        wk = stk.enter_context(tc.tile_pool(name="wk", bufs=4))
        # 3 aggregation PSUM tiles keep several chunks' selection-matmul
        # accumulations in flight across the gather/DVE/PE pipeline; ps_tr
        # drops to 1 buf (PSUM is 8 banks, bufs count per tag) — it only
        # serves layer 1's transpose + g-projection now that layer 0 reads
        # the host-precomputed x^T
        ps_ag = stk.enter_context(tc.tile_pool(name="ps_ag", bufs=4, space="PSUM"))
        ps_tr = stk.enter_context(tc.tile_pool(name="ps_tr", bufs=1, space="PSUM"))
        ps_h = stk.enter_context(tc.tile_pool(name="ps_h", bufs=2, space="PSUM"))

        # first-use safety: gather skips padded slots, so zero the message
        # buffers once (NaN * 0 selection would poison the PSUM accumulate)
        for _ in range(msg_bufs):
            m0 = msgp.tile([P, TMAX, D_HID], FP16, tag="msg")
            nc.vector.memset(m0[:], 0)

        gq = [0, 0]  # [queue counter, window counter (mod n_win)]

        def gather(out_ap, tab_ap, col0, n_idx, itile):
            if "gather" in ablate:
                # timing ablation: issue one tiny window per segment so the
                # tile framework still sees a write to the msg tile
                nc.gpsimd.dma_gather(
                    out_ap[:, 0:1, :],
                    tab_ap,
                    itile[:, col0 : col0 + 8],
                    num_idxs=128,
                    num_idxs_reg=128,
                    elem_size=D_HID,
                    queue_num=gq[0] % n_queues,
                )
                gq[0] += 1
                return
            for off in range(0, n_idx, maxi):
                n = min(maxi, n_idx - off)
                t0, t1 = off // P, (off + n) // P
                w = gq[1] % n_win
                if win_full is not None and win_full[w]:
                    nreg = n  # full on every core: compile-time count
                else:
                    nc.gpsimd.reg_load(
                        cnt_reg, gcnt_t[w % P : w % P + 1, w // P : w // P + 1]
                    )
                    nreg = cnt_reg
                nc.gpsimd.dma_gather(
                    out_ap[:, t0:t1, :],
                    tab_ap,
                    itile[:, col0 + off // 16 : col0 + (off + n) // 16],
                    num_idxs=n,
                    num_idxs_reg=nreg,
                    elem_size=D_HID,
                    queue_num=gq[0] % n_queues,
                )
                gq[0] += 1
                gq[1] += 1

        def accumulate(msg_t, agg_ps, til, T):
            """PE-accumulate the transposed selection aggregation for T tiles."""
            if "agg" in ablate:
                nc.vector.memset(agg_ps[:], 0)
                return
            if "nomm" in ablate:
                s_t = sp.tile([P, P], FP16, tag="S")
                nc.vector.tensor_scalar(
                    s_t[:], iota_h[:], dst_t[:, til : til + 1],
                    inv_t[:, til : til + 1],
                    mybir.AluOpType.is_equal, mybir.AluOpType.mult,
                )
                nc.tensor.matmul(
                    agg_ps[:], lhsT=msg_t[:, 0, :], rhs=s_t[:],
                    start=True, stop=True,
                )
                for t in range(1, T):
                    s_t = sp.tile([P, P], FP16, tag="S")
                    nc.vector.tensor_scalar(
                        s_t[:], iota_h[:], dst_t[:, til + t : til + t + 1],
                        inv_t[:, til + t : til + t + 1],
                        mybir.AluOpType.is_equal, mybir.AluOpType.mult,
                    )
                return
            for t in range(T):
                if "nos" in ablate:
                    s_t = ident  # constant tile; skips the DVE build
                else:
                    s_t = sp.tile([P, P], FP16, tag="S")
                    nc.vector.tensor_scalar(
                        s_t[:],
                        iota_h[:],
                        dst_t[:, til + t : til + t + 1],
                        inv_t[:, til + t : til + t + 1],
                        mybir.AluOpType.is_equal,
                        mybir.AluOpType.mult,
                    )
                nc.tensor.matmul(
                    agg_ps[:],
                    lhsT=msg_t[:, t, :],
                    rhs=s_t[:],
                    start=(t == 0),
                    stop=(t == T - 1),
                )

        def accumulate_packed(msg_t, agg_ps, til, T):
            """Layer-2 aggregation over the packed g table: each gathered
            256B row holds two nodes' 64-wide g rows; per-edge parity masks
            (baked into inve/invo) select the right half."""
            if "agg" in ablate:
                nc.vector.memset(agg_ps[:], 0)
                return
            for t in range(T):
                for par, ivt in ((0, inve_t), (1, invo_t)):
                    s_t = sp.tile([P, P], FP16, tag="S")
                    nc.vector.tensor_scalar(
                        s_t[:],
                        iota_h[:],
                        dst_t[:, til + t : til + t + 1],
                        ivt[:, til + t : til + t + 1],
                        mybir.AluOpType.is_equal,
                        mybir.AluOpType.mult,
                    )
                    nc.tensor.matmul(
                        agg_ps[0:64, :],
                        lhsT=msg_t[:, t, par * 64 : par * 64 + 64],
                        rhs=s_t[:],
                        start=(t == 0 and par == 0),
                        stop=(t == T - 1 and par == 1),
                    )

        ACT = mybir.ActivationFunctionType

        def fire_allgather(src_ap, dst_tile):
            # collectives must stay on the gpsimd queue (NRT straight-line
            # ordering guarantee)
            nc.gpsimd.collective_compute(
                "AllGather",
                mybir.AluOpType.bypass,
                replica_groups=[list(range(cfg.NCORES))],
                ins=[src_ap],
                outs=[dst_tile.opt()],
            )

        def run_layers(rep):
          h0_own, h0_full, g_own, h1T_own, g_full = alloc_tables(rep)
          for layer in range(3):
            dout = D_HID if layer < 2 else D_OUT
            packed = layer == 2
            if layer == 0:
                tab_lo, tab_hi = xtab[0:HALF, :], xtab[HALF:N, :]
            elif layer == 1:
                tab_lo, tab_hi = h0_full[0][:], h0_full[1][:]
            else:
                tab_lo, tab_hi = g_full[0][:], g_full[1][:]
            itile = idx2_t if packed else idx_t
            acc = accumulate_packed if packed else accumulate
            arows = 64 if packed else P  # valid partition rows of agg tiles
            wl_t = w_t[f"wl{layer}"]
            wr_t = w_t[f"wr{layer}"]

            # ---- pass A: low-half partial aggregation per chunk ----
            col = 0
            til = 0
            for c in range(NCH):
                Tl = T_low[c]
                if Tl == 0:
                    nc.vector.memset(loPart[c][:], 0)
                    continue
                msg_t = msgp.tile([P, TMAX, D_HID], FP16, tag="msg")
                gather(msg_t[:, :Tl, :], tab_lo, col, Tl * P, itile)
                agg_ps = ps_ag.tile([P, P], F32, tag="agg")
                acc(msg_t, agg_ps, til, Tl)
                nc.scalar.activation(
                    loPart[c][:arows, :], agg_ps[:arows, :], ACT.Copy
                )
                col += Tl * 8
                til += Tl

            # ---- pass B: high-half + combine + project ----
            for c in range(NCH):
                Th = T_high[c]
                nrows = min(P, NPC - c * P)

                hi_sb = wk.tile([P, P], FP16, tag="hi_sb")
                if Th:
                    msg_t = msgp.tile([P, TMAX, D_HID], FP16, tag="msg")
                    gather(msg_t[:, :Th, :], tab_hi, col, Th * P, itile)
                    agg_ps = ps_ag.tile([P, P], F32, tag="agg")
                    acc(msg_t, agg_ps, til, Th)
                    nc.scalar.activation(
                        hi_sb[:arows, :], agg_ps[:arows, :], ACT.Copy
                    )
                    col += Th * 8
                    til += Th
                else:
                    nc.vector.memset(hi_sb[:], 0)

                # root-term operand: own rows feature-major.  Layer 0 reads
                # the host-precomputed x^T and layer 2 the feature-major
                # relu(h1) written by layer 1; only layer 1 needs a PE
                # transpose of the node-major h0 rows.
                dmae = nc.sync if c % 2 == 0 else nc.scalar
                xT = wk.tile([P, P], FP16, tag="xT_sb")
                if layer == 1:
                    own_sb = wk.tile([P, D_HID], FP16, tag="own")
                    if nrows < P:
                        nc.vector.memset(own_sb[:], 0)
                    dmae.dma_start(
                        own_sb[:nrows], h0_own[c * P : c * P + nrows, :]
                    )
                    xT_ps = ps_tr.tile([P, P], FP16, tag="xT")
                    nc.tensor.transpose(xT_ps[:], own_sb[:], ident[:])
                    nc.scalar.activation(xT[:], xT_ps[:], ACT.Copy)
                else:
                    ownT = xownT[:] if layer == 0 else h1T_own[:]
                    if nrows < P:
                        nc.vector.memset(xT[:], 0)
                    dmae.dma_start(xT[:, :nrows], ownT[:, c * P : c * P + nrows])

                h_ps = ps_h.tile([P, D_HID], F32, tag="h")
                if layer == 0:
                    # h0 = relu((lo + hi) @ wl0 + x @ wr0): node-major
                    nc.tensor.matmul(h_ps[:, :dout], lhsT=loPart[c][:], rhs=wl_t[:], start=True, stop=False)
                    nc.tensor.matmul(h_ps[:, :dout], lhsT=hi_sb[:], rhs=wl_t[:], start=False, stop=False)
                    nc.tensor.matmul(h_ps[:, :dout], lhsT=xT[:], rhs=wr_t[:], start=False, stop=True)
                    h_sb = wk.tile([P, dout], FP16, tag="h_sb")
                    nc.scalar.activation(h_sb[:], h_ps[:, :dout], ACT.Relu)
                    nc.sync.dma_start(
                        h0_own[c * P : c * P + nrows, :], h_sb[:nrows]
                    )
                elif layer == 1:
                    # h1T = wl1^T (lo + hi) + wr1^T x^T: feature-major
                    nc.tensor.matmul(h_ps[:, :P], lhsT=wl_t[:], rhs=loPart[c][:], start=True, stop=False)
                    nc.tensor.matmul(h_ps[:, :P], lhsT=wl_t[:], rhs=hi_sb[:], start=False, stop=False)
                    nc.tensor.matmul(h_ps[:, :P], lhsT=wr_t[:], rhs=xT[:], start=False, stop=True)
                    h1T_sb = wk.tile([P, P], FP16, tag="h_sb")
                    nc.scalar.activation(h1T_sb[:], h_ps[:, :P], ACT.Relu)
                    nc.sync.dma_start(
                        h1T_own[:, c * P : c * P + nrows], h1T_sb[:, :nrows]
                    )
                    # g = relu(h1) @ wl2, the 64-wide boundary-1 payload
                    g_ps = ps_tr.tile([P, D_OUT], F32, tag="g")
                    nc.tensor.matmul(g_ps[:], lhsT=h1T_sb[:], rhs=w_t["wl2"][:], start=True, stop=True)
                    g_sb = wk.tile([P, D_OUT], FP16, tag="g_sb")
                    nc.scalar.activation(g_sb[:], g_ps[:], ACT.Copy)
                    nc.sync.dma_start(
                        g_own[c * P : c * P + nrows, :], g_sb[:nrows]
                    )
                else:
                    # h2 = (lo_g + hi_g)^T + relu(h1) @ wr2 (wl2 already
                    # applied before the boundary; identity collapses the
                    # transposed aggregate)
                    nc.tensor.matmul(h_ps[:, :dout], lhsT=loPart[c][0:64, :], rhs=ident[0:64, 0:64], start=True, stop=False)
                    nc.tensor.matmul(h_ps[:, :dout], lhsT=hi_sb[0:64, :], rhs=ident[0:64, 0:64], start=False, stop=False)
                    nc.tensor.matmul(h_ps[:, :dout], lhsT=xT[:], rhs=wr_t[:], start=False, stop=True)
                    o_sb = wk.tile([P, dout], F32, tag="o_sb")
                    nc.scalar.activation(o_sb[:], h_ps[:, :dout], ACT.Copy)
                    nc.sync.dma_start(out_d[c * P : c * P + nrows, :], o_sb[:nrows])

                # fire the low-half AllGather as soon as its producer rows
                # are done; the high-half one at end of layer
                if not skip_collectives:
                    if layer == 0:
                        if c == C_LOW_DONE:
                            fire_allgather(h0_own[0:HALFR, :], h0_full[0])
                        elif c == NCH - 1:
                            fire_allgather(h0_own[HALFR:NPC, :], h0_full[1])
                    elif layer == 1:
                        if c == C_LOW_DONE:
                            fire_allgather(g_own[0:HALFR, :], g_full[0])
                        elif c == NCH - 1:
                            fire_allgather(g_own[HALFR:NPC, :], g_full[1])

        for _rep in range(repeat):
            run_layers(_rep)

    nc.compile()
    return nc


def make_in_maps(cfg: GSCfg, inputs: dict, pre, has_bias):
    (T_low, T_high, idx16_sb, dstloc_sb, invd_sb, idx2_sb, inve_sb, invo_sb,
     gcnt_sb, _win_full) = pre
    x = np.asarray(inputs["x"], dtype=np.float32)
    x_h = x.astype(NP_FP16)
    xtab = table_permute(cfg, x_h)
    in_maps = []
    for i in range(cfg.NCORES):
        m = {
            "xtab": xtab,
            "xownT": np.ascontiguousarray(x_h[i * cfg.NPC : (i + 1) * cfg.NPC].T),
            "idx16": idx16_sb[i],
            "dstloc": dstloc_sb[i],
            "invd": invd_sb[i],
            "idx16l2": idx2_sb[i],
            "inve": inve_sb[i],
            "invo": invo_sb[i],
            "gcnt": gcnt_sb[i],
        }
        for li in range(3):
            m[f"wl{li}"] = np.asarray(inputs[f"w_l{li}"], np.float32).astype(NP_FP16)
            m[f"wr{li}"] = np.asarray(inputs[f"w_r{li}"], np.float32).astype(NP_FP16)
            if has_bias:
                b = np.asarray(inputs[f"b{li}"], dtype=np.float32)
                m[f"b{li}"] = np.tile(b[None, :], (P, 1))
        in_maps.append(m)
    return in_maps


def run(cfg: GSCfg, inputs: dict, trace: bool = False, tmpdir: str | None = None):
    """Preprocess, build, and run on the 8 cores; returns (out, results)."""
    ei = np.asarray(inputs["edge_index"])
    src = ei[0].astype(np.int64)
    dst = ei[1].astype(np.int64)

    pre = preprocess(cfg, src, dst)

    biases = [np.asarray(inputs[f"b{i}"], dtype=np.float32) for i in range(3)]
    has_bias = any(np.any(b != 0) for b in biases)

    cfg.win_full = pre[9]
    nc = build_program(cfg, pre[0], pre[1], has_bias)
    in_maps = make_in_maps(cfg, inputs, pre, has_bias)

    results = run_bass_kernel_spmd(
        nc,
        in_maps,
        core_ids=list(range(cfg.NCORES)),
        trace=trace,
        tmpdir=tmpdir,
    )
    outs = [np.asarray(r["out"], dtype=np.float32) for r in results.results]
    return np.concatenate(outs, axis=0), results


def kernel(**inputs) -> np.ndarray:
    cfg = GSCfg(n_nodes=50000, n_cores=8, d_in=128, d_hid=128, d_out=64, half=25000)
    out, _ = run(cfg, inputs, trace=False)
    return out


if __name__ == "__main__":
    import jax

    sys.path.insert(0, os.path.dirname(os.path.abspath(__file__)))
    import reference

    # the reference must run on CPU (jax gather jitted on the neuron
    # backend crashes neuronx-cc)
    with jax.default_device(jax.devices("cpu")[0]):
        inputs = {k: np.asarray(v) for k, v in reference.setup_inputs().items()}
        expected = np.asarray(reference.reference(**inputs))
    actual = kernel(**inputs)
    err = np.abs(actual - expected)
    rel = np.linalg.norm(actual - expected) / np.linalg.norm(expected)
    print("max abs err", err.max(), "rel", rel)



# revision 23
# speedup vs baseline: 1.0292x; 1.0292x over previous
# GraphSAGE (3-layer, mean aggregation) on 8 Trainium2 NeuronCores.
#
# Sharding: nodes are split into 8 contiguous ranges (6250 per core); edges are
# partitioned by destination node so each core's scatter-adds stay local.  Each
# layer's input features are replicated to every core via AllGather (the x table
# for layer 0 is simply fed to every core), so the per-edge source gathers are
# local HBM reads.
#
# All tables / weights / PE operands are fp16 (full-rate on the PE, 2x on the
# DVE, and ~8x the mantissa precision of bf16).
#
# Gathers issue 512-index SWDGE instructions round-robin over 4 queues
# (measured ~2x on hardware vs one queue; the descriptor-generation ucode
# parallelizes across queues), with per-core true index counts passed through
# a register so SPMD padding generates no descriptors.  Edge slots are sorted
# by source row within each segment for HBM locality.
#
# The boundary after layer 1 carries g = relu(h1) @ wl2 (64-wide, packed two
# nodes per 256B gather row, selected by per-edge parity masks), halving that
# AllGather; layer 1 produces feature-major relu(h1) directly (transposed
# matmuls) so layer 2 needs no per-chunk PE transpose.
#
# Each layer runs in TWO PASSES so the inter-layer AllGathers overlap compute:
#   pass A: for each chunk of 128 destination nodes, gather the edges whose
#     sources live in the LOW half of the feature table and accumulate the
#     transposed partial mean  loPart[c][feat, dst]  on the PE (via per-tile
#     selection matrices S[e, j] = (dst_local[e] == j) / deg built on the DVE),
#     parking the partial in SBUF.
#   pass B: gather the HIGH-half edges, accumulate hiPart, then
#     h = relu(loPart^T w_l + hiPart^T w_l + own^T^T w_r (+ b)) with three
#     accumulating matmuls into one PSUM tile; the Activation engine applies
#     relu / drains PSUM copies.
# The low-half AllGather of layer l's output fires mid-way through pass B
# (after the chunks that produce table rows [0, HALFR) complete), and the
# high-half AllGather's latency is hidden by layer l+1's pass A, which only
# needs the low half.
#
# Gather slots are padded (SPMD-uniform tile counts) with index -1: the SWDGE
# ucode skips trailing negative indices, so padding costs no descriptors/HBM
# bandwidth. Selection-matrix columns for padded slots are 0 (dst_local = -1),
# so whatever data sits in the padded SBUF slots contributes nothing.
import os
import sys

import numpy as np

for _p in ("/opt/trn_rl_repo", "/root/.axon_site/_ro/trn_rl_repo"):
    if _p not in sys.path and os.path.isdir(_p):
        sys.path.append(_p)

from concourse import bacc, mybir, tile  # noqa: E402
from concourse.bass_utils import axon_active, run_bass_kernel_spmd  # noqa: E402
from concourse.masks import make_identity  # noqa: E402

P = 128
SORT_SLOTS = True  # sort gather slots by source row within each segment
FP16 = mybir.dt.float16
F32 = mybir.dt.float32
I16 = mybir.dt.int16
NP_FP16 = np.float16
MAXI = 512  # gather indices per SWDGE instruction (descriptor ring is 1024)


class GSCfg:
    """Static problem configuration (shapes shared by all cores)."""

    def __init__(self, n_nodes, n_cores, d_in, d_hid, d_out, half, maxi=MAXI):
        assert n_nodes % n_cores == 0
        self.MAXI = maxi
        self.N = n_nodes
        self.NCORES = n_cores
        self.NPC = n_nodes // n_cores  # nodes per core
        self.D_IN = d_in
        self.D_HID = d_hid
        self.D_OUT = d_out
        # Gather tables are split at row `half` so int16 indices can address
        # each piece. half % n_cores == 0; per-core producer split = HALFR.
        self.HALF = half
        assert half % n_cores == 0
        self.HALFR = half // n_cores
        assert self.HALFR < self.NPC
        assert half <= 2**15 and (n_nodes - half) < 2**15
        self.NCH = (self.NPC + P - 1) // P  # dst chunks per core


def preprocess(cfg: GSCfg, src: np.ndarray, dst: np.ndarray):
    """Partition + sort edges by destination, pad to an SPMD-uniform tile
    layout, and build the per-core SBUF-layout index/selection arrays."""
    N, NPC, NCH, HALF, HALFR = cfg.N, cfg.NPC, cfg.NCH, cfg.HALF, cfg.HALFR
    NC = cfg.NCORES

    deg = np.bincount(dst, minlength=N)
    invdeg_per_node = (1.0 / np.maximum(deg, 1)).astype(np.float32)

    order = np.argsort(dst, kind="stable")
    s_src = src[order]
    s_dst = dst[order]

    # edge index ranges for (core, chunk)
    chunk_lo = np.empty((NC, NCH), dtype=np.int64)
    chunk_hi = np.empty((NC, NCH), dtype=np.int64)
    for i in range(NC):
        for c in range(NCH):
            d0 = i * NPC + c * P
            d1 = i * NPC + min((c + 1) * P, NPC)
            chunk_lo[i, c] = np.searchsorted(s_dst, d0, side="left")
            chunk_hi[i, c] = np.searchsorted(s_dst, d1, side="left")

    # split each chunk's edges into low-src / high-src groups
    lo_cnt = np.zeros((NC, NCH), dtype=np.int64)
    for i in range(NC):
        for c in range(NCH):
            e0, e1 = chunk_lo[i, c], chunk_hi[i, c]
            lo_cnt[i, c] = int(np.count_nonzero((s_src[e0:e1] % NPC) < HALFR))
    hi_cnt = (chunk_hi - chunk_lo) - lo_cnt

    cdiv = lambda a, b: -(-a // b)
    T_low = [int(cdiv(int(lo_cnt[:, c].max()), P)) for c in range(NCH)]
    T_high = [int(cdiv(int(hi_cnt[:, c].max()), P)) for c in range(NCH)]
    TT = sum(T_low) + sum(T_high)
    SLOTS = TT * P

    # Padded slots carry index -1: the SWDGE ucode generates no descriptors
    # for a trailing negative run, and each core passes its true per-window
    # count through a register (num_idxs_reg), so padding costs no HBM
    # bandwidth.  Windows that are entirely padding keep one real index
    # (row 0, count 1) so the completion path always sees >= 1 descriptor.
    # dstloc=-1 masks padded slots out of the selection matrices either way.
    idx16 = np.full((NC, SLOTS), -1, dtype=np.int16)
    idx16_l2 = np.full((NC, SLOTS), -1, dtype=np.int16)  # packed-pair rows
    dstloc = np.full((NC, SLOTS), -1.0, dtype=np.float32)
    invd = np.zeros((NC, SLOTS), dtype=np.float32)
    inve = np.zeros((NC, SLOTS), dtype=np.float32)  # invd if even table row
    invo = np.zeros((NC, SLOTS), dtype=np.float32)  # invd if odd table row
    MX = cfg.MAXI
    n_win = sum(-(-(t * P) // MX) for t in T_low + T_high)
    gcnt = np.ones((NC, n_win), dtype=np.int32)
    # windows full on every core keep a compile-time count (no reg_load)
    win_full = np.ones(n_win, dtype=bool)

    # Layer order within the slot array: all chunks' LOW segments first
    # (pass A), then all chunks' HIGH segments (pass B).
    HIR = NPC - HALFR
    for i in range(NC):
        seg_data = {}
        for c in range(NCH):
            e0, e1 = chunk_lo[i, c], chunk_hi[i, c]
            seg_src = s_src[e0:e1]
            seg_dst = s_dst[e0:e1]
            s_i, s_r = seg_src // NPC, seg_src % NPC
            is_lo = s_r < HALFR
            tidx = np.where(is_lo, s_i * HALFR + s_r, s_i * HIR + (s_r - HALFR))
            base = i * NPC + c * P
            for half_sel in (0, 1):
                m = is_lo if half_sel == 0 else ~is_lo
                # source-sorted slot order: consecutive gather descriptors hit
                # nearby table rows (HBM row-buffer locality); any order within
                # a segment is valid since dstloc/invd follow the slot
                if SORT_SLOTS:
                    o = np.argsort(tidx[m], kind="stable")
                else:
                    o = np.arange(int(np.count_nonzero(m)))
                seg_data[(half_sel, c)] = (
                    tidx[m][o].astype(np.int16),
                    (seg_dst[m][o] - base).astype(np.float32),
                    invdeg_per_node[seg_dst[m][o]],
                )
        pos = 0
        win = 0
        for half_sel in (0, 1):
            T_pads = T_low if half_sel == 0 else T_high
            for c in range(NCH):
                ss, dd, vv = seg_data[(half_sel, c)]
                n = len(ss)
                t_pad = T_pads[c]
                assert n <= t_pad * P
                idx16[i, pos : pos + n] = ss
                idx16_l2[i, pos : pos + n] = ss // 2
                dstloc[i, pos : pos + n] = dd
                invd[i, pos : pos + n] = vv
                par = (ss & 1).astype(np.float32)
                inve[i, pos : pos + n] = vv * (1.0 - par)
                invo[i, pos : pos + n] = vv * par
                for w0 in range(0, t_pad * P, MX):
                    wlen = min(MX, t_pad * P - w0)
                    real = min(max(n - w0, 0), wlen)
                    if real == 0:
                        idx16[i, pos + w0] = 0
                        idx16_l2[i, pos + w0] = 0
                        real = 1
                    if real < wlen:
                        win_full[win] = False
                    gcnt[i, win] = real
                    win += 1
                pos += t_pad * P
        assert pos == SLOTS and win == n_win

    # SBUF layouts:
    #  idx16_sb [128, SLOTS//16]: per gather segment, slot j -> [j%16, j//16],
    #    replicated across the eight 16-partition groups.  Segments are
    #    multiples of 128 slots, so the per-segment wrap equals a global wrap.
    def idx_layout(a):
        w = a.reshape(NC, SLOTS // 16, 16).transpose(0, 2, 1)  # [NC,16,cols]
        return np.ascontiguousarray(np.tile(w, (1, 8, 1)))  # [NC,128,cols]

    #  dstloc/invd [128, TT]: slot j -> [j%128, j//128]
    def col_layout(a):
        return np.ascontiguousarray(a.reshape(NC, TT, P).transpose(0, 2, 1))

    #  gcnt [128, ceil(n_win/128)]: window w -> [w%128, w//128]
    wcols = -(-n_win // P)
    gcnt_sb = np.zeros((NC, P, wcols), dtype=np.int32)
    for w in range(n_win):
        gcnt_sb[:, w % P, w // P] = gcnt[:, w]

    return (
        T_low,
        T_high,
        idx_layout(idx16),
        col_layout(dstloc),
        col_layout(invd),
        idx_layout(idx16_l2),
        col_layout(inve),
        col_layout(invo),
        gcnt_sb,
        win_full,
    )


def table_permute(cfg: GSCfg, x: np.ndarray) -> np.ndarray:
    """Reorder node rows into the core-major-half gather-table layout."""
    g = np.arange(cfg.N)
    i, r = g // cfg.NPC, g % cfg.NPC
    hr, hir = cfg.HALFR, cfg.NPC - cfg.HALFR
    gp = np.where(r < hr, i * hr + r, cfg.HALF + i * hir + (r - hr))
    out = np.empty_like(x)
    out[gp] = x[g]
    return out


def build_program(cfg: GSCfg, T_low, T_high, has_bias, skip_collectives=False,
                  repeat=1, n_queues=4, ablate=(), msg_bufs=6, sel_bufs=12):
    """Build the SPMD Bass program (identical instruction stream per core).

    repeat>1 runs the whole 3-layer computation that many times inside one
    NEFF (constants stay resident); used by the benchmark harness to measure
    per-iteration device time as a slope, independent of dispatch overhead."""
    assert not has_bias, "bias path not implemented (reference biases are zero)"

    N, NPC, NCH, HALF, HALFR = cfg.N, cfg.NPC, cfg.NCH, cfg.HALF, cfg.HALFR
    D_IN, D_HID, D_OUT = cfg.D_IN, cfg.D_HID, cfg.D_OUT
    TT = sum(T_low) + sum(T_high)
    TMAX = max(max(T_low), max(T_high))
    HIR = NPC - HALFR
    # pass-B chunk index after which all of h_own[0:HALFR] has been written
    C_LOW_DONE = (HALFR + P - 1) // P - 1

    nc = bacc.Bacc(
        "TRN2",
        target_bir_lowering=False,
        debug=not axon_active(),
        num_devices=cfg.NCORES,
        num_swdge_queues=n_queues,
        dynamic_dma_scratch_size=max(16384, 32 * cfg.MAXI),
    )
    maxi = cfg.MAXI
    n_win = sum(-(-(t * P) // maxi) for t in T_low + T_high)
    wcols = -(-n_win // P)
    win_full = getattr(cfg, "win_full", None)

    xtab = nc.dram_tensor("xtab", [N, D_IN], FP16, kind="ExternalInput")
    # feature-major own rows (precomputed on host): layer 0's root term reads
    # columns directly, skipping the per-chunk PE transpose + PSUM drain
    xownT = nc.dram_tensor("xownT", [D_IN, NPC], FP16, kind="ExternalInput")
    idx_d = nc.dram_tensor("idx16", [P, TT * 8], I16, kind="ExternalInput")
    dst_d = nc.dram_tensor("dstloc", [P, TT], F32, kind="ExternalInput")
    inv_d = nc.dram_tensor("invd", [P, TT], F32, kind="ExternalInput")
    idx2_d = nc.dram_tensor("idx16l2", [P, TT * 8], I16, kind="ExternalInput")
    inve_d = nc.dram_tensor("inve", [P, TT], F32, kind="ExternalInput")
    invo_d = nc.dram_tensor("invo", [P, TT], F32, kind="ExternalInput")
    gcnt_d = nc.dram_tensor("gcnt", [P, wcols], mybir.dt.int32, kind="ExternalInput")
    w_d = {}
    for li, (din, dout) in enumerate(((D_IN, D_HID), (D_HID, D_HID), (D_HID, D_OUT))):
        w_d[f"wl{li}"] = nc.dram_tensor(f"wl{li}", [din, dout], FP16, kind="ExternalInput")
        w_d[f"wr{li}"] = nc.dram_tensor(f"wr{li}", [din, dout], FP16, kind="ExternalInput")

    out_d = nc.dram_tensor("out", [NPC, D_OUT], F32, kind="ExternalOutput")

    from contextlib import ExitStack

    with tile.TileContext(nc) as tc, ExitStack() as stk:
        # ---- constants / static SBUF residents ----
        const = stk.enter_context(tc.tile_pool(name="const", bufs=1))
        iota_i = const.tile([P, P], mybir.dt.int32, name="iota_i")
        nc.gpsimd.iota(iota_i[:], pattern=[[1, P]], base=0, channel_multiplier=0)
        iota_h = const.tile([P, P], FP16, name="iota_h")
        nc.vector.tensor_copy(iota_h[:], iota_i[:])
        ident = const.tile([P, P], FP16, name="ident")
        make_identity(nc, ident[:])

        idx_t = const.tile([P, TT * 8], I16, name="idx_t")
        nc.sync.dma_start(idx_t[:], idx_d[:])
        dst_t = const.tile([P, TT], F32, name="dst_t")
        nc.sync.dma_start(dst_t[:], dst_d[:])
        inv_t = const.tile([P, TT], F32, name="inv_t")
        nc.sync.dma_start(inv_t[:], inv_d[:])
        idx2_t = const.tile([P, TT * 8], I16, name="idx2_t")
        nc.sync.dma_start(idx2_t[:], idx2_d[:])
        inve_t = const.tile([P, TT], F32, name="inve_t")
        nc.sync.dma_start(inve_t[:], inve_d[:])
        invo_t = const.tile([P, TT], F32, name="invo_t")
        nc.sync.dma_start(invo_t[:], invo_d[:])
        gcnt_t = const.tile([P, wcols], mybir.dt.int32, name="gcnt_t")
        nc.sync.dma_start(gcnt_t[:], gcnt_d[:])
        cnt_reg = nc.gpsimd.alloc_register("gather_cnt")

        w_t = {}
        for k, d in w_d.items():
            w_t[k] = const.tile(list(d.shape), d.dtype, name=f"{k}_t")
            nc.sync.dma_start(w_t[k][:], d[:])

        # per-chunk low-half partial aggregations (persistent across a layer)
        lop = stk.enter_context(tc.tile_pool(name="lop", bufs=1))
        loPart = [lop.tile([P, P], FP16, name=f"loPart{c}") for c in range(NCH)]

        # ---- inter-layer DRAM tables ----
        # (allocated per repetition: Shared collective outputs allow only a
        # single writer instruction)
        dram = stk.enter_context(tc.tile_pool(name="dram", bufs=1, space="DRAM"))

        def alloc_tables(rep):
            # layer-0 output: node-major h0 rows + its AllGather halves
            h0_own = dram.tile([NPC, D_HID], FP16, name=f"h0_own_r{rep}")
            h0_full = (
                dram.tile([HALF, D_HID], FP16, name=f"h0_full0_r{rep}",
                          addr_space="Shared"),
                dram.tile([N - HALF, D_HID], FP16, name=f"h0_full1_r{rep}",
                          addr_space="Shared"),
            )
            # layer-1 outputs: g = relu(h1) @ wl2 rows (node-major, feeds the
            # boundary-1 AllGather; consumed packed two-nodes-per-row) and
            # feature-major relu(h1) for layer 2's root term
            g_own = dram.tile([NPC, D_OUT], FP16, name=f"g_own_r{rep}")
            h1T_own = dram.tile([P, NPC], FP16, name=f"h1T_own_r{rep}")
            g_full = (
                dram.tile([HALF // 2, P], FP16, name=f"g_full0_r{rep}",
                          addr_space="Shared"),
                dram.tile([(N - HALF) // 2, P], FP16, name=f"g_full1_r{rep}",
                          addr_space="Shared"),
            )
            return h0_own, h0_full, g_own, h1T_own, g_full

        # ---- working pools ----
        msgp = stk.enter_context(tc.tile_pool(name="msg", bufs=msg_bufs))
        sp = stk.enter_context(tc.tile_pool(name="sel", bufs=sel_bufs))
        wk = stk.enter_context(tc.tile_pool(name="wk", bufs=3))
        # 3 aggregation PSUM tiles keep several chunks' selection-matmul
        # accumulations in flight across the gather/DVE/PE pipeline; ps_tr
        # drops to 1 buf (PSUM is 8 banks, bufs count per tag) — it only
        # serves layer 1's transpose + g-projection now that layer 0 reads
        # the host-precomputed x^T
        ps_ag = stk.enter_context(tc.tile_pool(name="ps_ag", bufs=4, space="PSUM"))
        ps_tr = stk.enter_context(tc.tile_pool(name="ps_tr", bufs=1, space="PSUM"))
        ps_h = stk.enter_context(tc.tile_pool(name="ps_h", bufs=2, space="PSUM"))

        # first-use safety: gather skips padded slots, so zero the message
        # buffers once (NaN * 0 selection would poison the PSUM accumulate)
        for _ in range(msg_bufs):
            m0 = msgp.tile([P, TMAX, D_HID], FP16, tag="msg")
            nc.vector.memset(m0[:], 0)

        gq = [0, 0]  # [queue counter, window counter (mod n_win)]

        def gather(out_ap, tab_ap, col0, n_idx, itile):
            if "gather" in ablate:
                # timing ablation: issue one tiny window per segment so the
                # tile framework still sees a write to the msg tile
                nc.gpsimd.dma_gather(
                    out_ap[:, 0:1, :],
                    tab_ap,
                    itile[:, col0 : col0 + 8],
                    num_idxs=128,
                    num_idxs_reg=128,
                    elem_size=D_HID,
                    queue_num=gq[0] % n_queues,
                )
                gq[0] += 1
                return
            for off in range(0, n_idx, maxi):
                n = min(maxi, n_idx - off)
                t0, t1 = off // P, (off + n) // P
                w = gq[1] % n_win
                if win_full is not None and win_full[w]:
                    nreg = n  # full on every core: compile-time count
                else:
                    nc.gpsimd.reg_load(
                        cnt_reg, gcnt_t[w % P : w % P + 1, w // P : w // P + 1]
                    )
                    nreg = cnt_reg
                nc.gpsimd.dma_gather(
                    out_ap[:, t0:t1, :],
                    tab_ap,
                    itile[:, col0 + off // 16 : col0 + (off + n) // 16],
                    num_idxs=n,
                    num_idxs_reg=nreg,
                    elem_size=D_HID,
                    queue_num=gq[0] % n_queues,
                )
                gq[0] += 1
                gq[1] += 1

        def accumulate(msg_t, agg_ps, til, T):
            """PE-accumulate the transposed selection aggregation for T tiles."""
            if "agg" in ablate:
                nc.vector.memset(agg_ps[:], 0)
                return
            if "nomm" in ablate:
                s_t = sp.tile([P, P], FP16, tag="S")
                nc.vector.tensor_scalar(
                    s_t[:], iota_h[:], dst_t[:, til : til + 1],
                    inv_t[:, til : til + 1],
                    mybir.AluOpType.is_equal, mybir.AluOpType.mult,
                )
                nc.tensor.matmul(
                    agg_ps[:], lhsT=msg_t[:, 0, :], rhs=s_t[:],
                    start=True, stop=True,
                )
                for t in range(1, T):
                    s_t = sp.tile([P, P], FP16, tag="S")
                    nc.vector.tensor_scalar(
                        s_t[:], iota_h[:], dst_t[:, til + t : til + t + 1],
                        inv_t[:, til + t : til + t + 1],
                        mybir.AluOpType.is_equal, mybir.AluOpType.mult,
                    )
                return
            for t in range(T):
                if "nos" in ablate:
                    s_t = ident  # constant tile; skips the DVE build
                else:
                    s_t = sp.tile([P, P], FP16, tag="S")
                    nc.vector.tensor_scalar(
                        s_t[:],
                        iota_h[:],
                        dst_t[:, til + t : til + t + 1],
                        inv_t[:, til + t : til + t + 1],
                        mybir.AluOpType.is_equal,
                        mybir.AluOpType.mult,
                    )
                nc.tensor.matmul(
                    agg_ps[:],
                    lhsT=msg_t[:, t, :],
                    rhs=s_t[:],
                    start=(t == 0),
                    stop=(t == T - 1),
                )

        def accumulate_packed(msg_t, agg_ps, til, T):
            """Layer-2 aggregation over the packed g table: each gathered
            256B row holds two nodes' 64-wide g rows; per-edge parity masks
            (baked into inve/invo) select the right half."""
            if "agg" in ablate:
                nc.vector.memset(agg_ps[:], 0)
                return
            for t in range(T):
                for par, ivt in ((0, inve_t), (1, invo_t)):
                    s_t = sp.tile([P, P], FP16, tag="S")
                    nc.vector.tensor_scalar(
                        s_t[:],
                        iota_h[:],
                        dst_t[:, til + t : til + t + 1],
                        ivt[:, til + t : til + t + 1],
                        mybir.AluOpType.is_equal,
                        mybir.AluOpType.mult,
                    )
                    nc.tensor.matmul(
                        agg_ps[0:64, :],
                        lhsT=msg_t[:, t, par * 64 : par * 64 + 64],
                        rhs=s_t[:],
                        start=(t == 0 and par == 0),
                        stop=(t == T - 1 and par == 1),
                    )

        ACT = mybir.ActivationFunctionType

        def fire_allgather(src_ap, dst_tile):
            # collectives must stay on the gpsimd queue (NRT straight-line
            # ordering guarantee)
            nc.gpsimd.collective_compute(
                "AllGather",
                mybir.AluOpType.bypass,
                replica_groups=[list(range(cfg.NCORES))],
                ins=[src_ap],
                outs=[dst_tile.opt()],
            )

        def run_layers(rep):
          h0_own, h0_full, g_own, h1T_own, g_full = alloc_tables(rep)
          for layer in range(3):
            dout = D_HID if layer < 2 else D_OUT
            packed = layer == 2
            if layer == 0:
                tab_lo, tab_hi = xtab[0:HALF, :], xtab[HALF:N, :]
            elif layer == 1:
                tab_lo, tab_hi = h0_full[0][:], h0_full[1][:]
            else:
                tab_lo, tab_hi = g_full[0][:], g_full[1][:]
            itile = idx2_t if packed else idx_t
            acc = accumulate_packed if packed else accumulate
            arows = 64 if packed else P  # valid partition rows of agg tiles
            wl_t = w_t[f"wl{layer}"]
            wr_t = w_t[f"wr{layer}"]

            # ---- pass A: low-half partial aggregation per chunk ----
            col = 0
            til = 0
            for c in range(NCH):
                Tl = T_low[c]
                if Tl == 0:
                    nc.vector.memset(loPart[c][:], 0)
                    continue
                msg_t = msgp.tile([P, TMAX, D_HID], FP16, tag="msg")
                gather(msg_t[:, :Tl, :], tab_lo, col, Tl * P, itile)
                agg_ps = ps_ag.tile([P, P], F32, tag="agg")
                acc(msg_t, agg_ps, til, Tl)
                nc.scalar.activation(
                    loPart[c][:arows, :], agg_ps[:arows, :], ACT.Copy
                )
                col += Tl * 8
                til += Tl

            # ---- pass B: high-half + combine + project ----
            for c in range(NCH):
                Th = T_high[c]
                nrows = min(P, NPC - c * P)

                hi_sb = wk.tile([P, P], FP16, tag="hi_sb")
                if Th:
                    msg_t = msgp.tile([P, TMAX, D_HID], FP16, tag="msg")
                    gather(msg_t[:, :Th, :], tab_hi, col, Th * P, itile)
                    agg_ps = ps_ag.tile([P, P], F32, tag="agg")
                    acc(msg_t, agg_ps, til, Th)
                    nc.scalar.activation(
                        hi_sb[:arows, :], agg_ps[:arows, :], ACT.Copy
                    )
                    col += Th * 8
                    til += Th
                else:
                    nc.vector.memset(hi_sb[:], 0)

                # root-term operand: own rows feature-major.  Layer 0 reads
                # the host-precomputed x^T and layer 2 the feature-major
                # relu(h1) written by layer 1; only layer 1 needs a PE
                # transpose of the node-major h0 rows.
                dmae = nc.sync if c % 2 == 0 else nc.scalar
                xT = wk.tile([P, P], FP16, tag="xT_sb")
                if layer == 1:
                    own_sb = wk.tile([P, D_HID], FP16, tag="own")
                    if nrows < P:
                        nc.vector.memset(own_sb[:], 0)
                    dmae.dma_start(
                        own_sb[:nrows], h0_own[c * P : c * P + nrows, :]
                    )
                    xT_ps = ps_tr.tile([P, P], FP16, tag="xT")
                    nc.tensor.transpose(xT_ps[:], own_sb[:], ident[:])
                    nc.scalar.activation(xT[:], xT_ps[:], ACT.Copy)
                else:
                    ownT = xownT[:] if layer == 0 else h1T_own[:]
                    if nrows < P:
                        nc.vector.memset(xT[:], 0)
                    dmae.dma_start(xT[:, :nrows], ownT[:, c * P : c * P + nrows])

                h_ps = ps_h.tile([P, D_HID], F32, tag="h")
                if layer == 0:
                    # h0 = relu((lo + hi) @ wl0 + x @ wr0): node-major
                    nc.tensor.matmul(h_ps[:, :dout], lhsT=loPart[c][:], rhs=wl_t[:], start=True, stop=False)
                    nc.tensor.matmul(h_ps[:, :dout], lhsT=hi_sb[:], rhs=wl_t[:], start=False, stop=False)
                    nc.tensor.matmul(h_ps[:, :dout], lhsT=xT[:], rhs=wr_t[:], start=False, stop=True)
                    h_sb = wk.tile([P, dout], FP16, tag="h_sb")
                    nc.scalar.activation(h_sb[:], h_ps[:, :dout], ACT.Relu)
                    nc.sync.dma_start(
                        h0_own[c * P : c * P + nrows, :], h_sb[:nrows]
                    )
                elif layer == 1:
                    # h1T = wl1^T (lo + hi) + wr1^T x^T: feature-major
                    nc.tensor.matmul(h_ps[:, :P], lhsT=wl_t[:], rhs=loPart[c][:], start=True, stop=False)
                    nc.tensor.matmul(h_ps[:, :P], lhsT=wl_t[:], rhs=hi_sb[:], start=False, stop=False)
                    nc.tensor.matmul(h_ps[:, :P], lhsT=wr_t[:], rhs=xT[:], start=False, stop=True)
                    h1T_sb = wk.tile([P, P], FP16, tag="h_sb")
                    nc.scalar.activation(h1T_sb[:], h_ps[:, :P], ACT.Relu)
                    nc.sync.dma_start(
                        h1T_own[:, c * P : c * P + nrows], h1T_sb[:, :nrows]
                    )
                    # g = relu(h1) @ wl2, the 64-wide boundary-1 payload
                    g_ps = ps_tr.tile([P, D_OUT], F32, tag="g")
                    nc.tensor.matmul(g_ps[:], lhsT=h1T_sb[:], rhs=w_t["wl2"][:], start=True, stop=True)
                    g_sb = wk.tile([P, D_OUT], FP16, tag="g_sb")
                    nc.scalar.activation(g_sb[:], g_ps[:], ACT.Copy)
                    nc.sync.dma_start(
                        g_own[c * P : c * P + nrows, :], g_sb[:nrows]
                    )
                else:
                    # h2 = (lo_g + hi_g)^T + relu(h1) @ wr2 (wl2 already
                    # applied before the boundary; identity collapses the
                    # transposed aggregate)
                    nc.tensor.matmul(h_ps[:, :dout], lhsT=loPart[c][0:64, :], rhs=ident[0:64, 0:64], start=True, stop=False)
                    nc.tensor.matmul(h_ps[:, :dout], lhsT=hi_sb[0:64, :], rhs=ident[0:64, 0:64], start=False, stop=False)
                    nc.tensor.matmul(h_ps[:, :dout], lhsT=xT[:], rhs=wr_t[:], start=False, stop=True)
                    o_sb = wk.tile([P, dout], F32, tag="o_sb")
                    nc.scalar.activation(o_sb[:], h_ps[:, :dout], ACT.Copy)
                    nc.sync.dma_start(out_d[c * P : c * P + nrows, :], o_sb[:nrows])

                # fire the low-half AllGather as soon as its producer rows
                # are done; the high-half one at end of layer
                if not skip_collectives:
                    if layer == 0:
                        if c == C_LOW_DONE:
                            fire_allgather(h0_own[0:HALFR, :], h0_full[0])
                        elif c == NCH - 1:
                            fire_allgather(h0_own[HALFR:NPC, :], h0_full[1])
                    elif layer == 1:
                        if c == C_LOW_DONE:
                            fire_allgather(g_own[0:HALFR, :], g_full[0])
                        elif c == NCH - 1:
                            fire_allgather(g_own[HALFR:NPC, :], g_full[1])

        for _rep in range(repeat):
            run_layers(_rep)

    nc.compile()
    return nc


def make_in_maps(cfg: GSCfg, inputs: dict, pre, has_bias):
    (T_low, T_high, idx16_sb, dstloc_sb, invd_sb, idx2_sb, inve_sb, invo_sb,
     gcnt_sb, _win_full) = pre
    x = np.asarray(inputs["x"], dtype=np.float32)
    x_h = x.astype(NP_FP16)
    xtab = table_permute(cfg, x_h)
    in_maps = []
    for i in range(cfg.NCORES):
        m = {
            "xtab": xtab,
            "xownT": np.ascontiguousarray(x_h[i * cfg.NPC : (i + 1) * cfg.NPC].T),
            "idx16": idx16_sb[i],
            "dstloc": dstloc_sb[i],
            "invd": invd_sb[i],
            "idx16l2": idx2_sb[i],
            "inve": inve_sb[i],
            "invo": invo_sb[i],
            "gcnt": gcnt_sb[i],
        }
        for li in range(3):
            m[f"wl{li}"] = np.asarray(inputs[f"w_l{li}"], np.float32).astype(NP_FP16)
            m[f"wr{li}"] = np.asarray(inputs[f"w_r{li}"], np.float32).astype(NP_FP16)
            if has_bias:
                b = np.asarray(inputs[f"b{li}"], dtype=np.float32)
                m[f"b{li}"] = np.tile(b[None, :], (P, 1))
        in_maps.append(m)
    return in_maps


def run(cfg: GSCfg, inputs: dict, trace: bool = False, tmpdir: str | None = None):
    """Preprocess, build, and run on the 8 cores; returns (out, results)."""
    ei = np.asarray(inputs["edge_index"])
    src = ei[0].astype(np.int64)
    dst = ei[1].astype(np.int64)

    pre = preprocess(cfg, src, dst)

    biases = [np.asarray(inputs[f"b{i}"], dtype=np.float32) for i in range(3)]
    has_bias = any(np.any(b != 0) for b in biases)

    cfg.win_full = pre[9]
    nc = build_program(cfg, pre[0], pre[1], has_bias)
    in_maps = make_in_maps(cfg, inputs, pre, has_bias)

    results = run_bass_kernel_spmd(
        nc,
        in_maps,
        core_ids=list(range(cfg.NCORES)),
        trace=trace,
        tmpdir=tmpdir,
    )
    outs = [np.asarray(r["out"], dtype=np.float32) for r in results.results]
    return np.concatenate(outs, axis=0), results


def kernel(**inputs) -> np.ndarray:
    cfg = GSCfg(n_nodes=50000, n_cores=8, d_in=128, d_hid=128, d_out=64, half=25000)
    out, _ = run(cfg, inputs, trace=False)
    return out


if __name__ == "__main__":
    import jax

    sys.path.insert(0, os.path.dirname(os.path.abspath(__file__)))
    import reference

    # the reference must run on CPU (jax gather jitted on the neuron
    # backend crashes neuronx-cc)
    with jax.default_device(jax.devices("cpu")[0]):
        inputs = {k: np.asarray(v) for k, v in reference.setup_inputs().items()}
        expected = np.asarray(reference.reference(**inputs))
    actual = kernel(**inputs)
    err = np.abs(actual - expected)
    rel = np.linalg.norm(actual - expected) / np.linalg.norm(expected)
    print("max abs err", err.max(), "rel", rel)

